# revision 9
# baseline (speedup 1.0000x reference)
"""Batched Viterbi decode (CRF) on 8 Trainium2 NeuronCores.

Problem (hardcoded): B=64, T=2048, S=128 tags.
  transitions [64, 129, 128] f32   (row 128 = start scores)
  emissions   [64, 2048, 128] f32
  lengths     [64] i32
  output z    [64, 2048] i32  (Viterbi path, zeros past each length)

Strategy:
  - Data-parallel over batch: core c owns sequences 8c..8c+7.
  - Device computes the forward max-plus DP *values* only (no argmax):
      t1_t[i] = max_j(t1_{t-1}[j] + trans[j,i]) + em[t,i]
    keeping the full t1 history [128, T*8] in SBUF, streamed to DRAM at the
    end. All adds/maxes are IEEE fp32 in the same association order as the
    reference, so the t1 history is bitwise identical to the reference's.
  - Host reconstructs the backpointers only along the surviving path
    (argmax_j of one 128-column per step per sequence), bitwise-reproducing
    the reference's tie-breaking, then emits z.

  Per step the device does (per core, 8 sequences):
    scores_s[j,i] = trans_s[j,i] + t1_col_s[j]   (6 seqs on ACT bias-add,
                                                  2 seqs on DVE tensor_scalar)
    scoresT_s = PE transpose(scores_s) -> PSUM  (per 4-seq group tensors)
    m[g] = DVE tensor_reduce(max) over j for each 4-seq group
    t1hist[:, t*8+g*4 : ...] = m[g] + em_cols[g]  (DVE tensor_tensor)
"""

import os
from contextlib import ExitStack

import numpy as np

S = 128
T = 2048
NS = 8          # sequences per core
N_CORES = 8
B = NS * N_CORES
ADD_ENG = "aavaavav"   # per-seq engine for the t1 broadcast add: a=ACT, v=DVE

_CACHE = {}


def _build_forward():
    import concourse.bacc as bacc
    import concourse.mybir as mybir
    import concourse.tile as tile

    F32 = mybir.dt.float32
    nc = bacc.Bacc("TRN2", num_devices=N_CORES)
    trans_in = nc.dram_tensor("transitions", [NS, S + 1, S], F32, kind="ExternalInput")
    em_in = nc.dram_tensor("emissions", [NS, T, S], F32, kind="ExternalInput")
    ident_in = nc.dram_tensor("identity", [S, S], F32, kind="ExternalInput")
    t1_out = nc.dram_tensor("t1hist", [S, T * NS], F32, kind="ExternalOutput")

    with ExitStack() as ctx:
        trans_sb = ctx.enter_context(nc.sbuf_tensor([S, NS * S], F32))
        em_cols = ctx.enter_context(nc.sbuf_tensor([S, T * NS], F32))
        t1hist = ctx.enter_context(nc.sbuf_tensor([S, T * NS], F32))
        ident = ctx.enter_context(nc.sbuf_tensor([S, S], F32))
        start_sb = ctx.enter_context(nc.sbuf_tensor([S, NS], F32))
        em0_sb = ctx.enter_context(nc.sbuf_tensor([S, NS], F32))
        # per-buf 2-bank PSUM tensors: all 8 transposes land in one tensor so
        # a single reduce + single em-add serve the whole step
        psum_pp = [
            ctx.enter_context(nc.psum_tensor(f"pspp{k}", [S, NS * S], F32))
            for k in range(3)
        ]

        with tile.TileContext(nc) as tc, ExitStack() as pctx:
            sc_pool = pctx.enter_context(tc.tile_pool(name="scores", bufs=3))
            tmp_pool = pctx.enter_context(tc.tile_pool(name="tmp", bufs=3))
            stage_pool = pctx.enter_context(tc.tile_pool(name="stage", bufs=4))
            pst_pool = pctx.enter_context(tc.tile_pool(name="pst", bufs=2, space="PSUM"))

            # ---- prologue: transitions, identity, t1_0 ----
            for s in range(NS):
                nc.sync.dma_start(trans_sb[:, s * S:(s + 1) * S], trans_in[s, 0:S, :])
            nc.sync.dma_start(ident[:], ident_in[:])
            for s in range(NS):
                nc.sync.dma_start(
                    start_sb[:, s:s + 1], trans_in[s, S:S + 1, :].rearrange("o p -> p o")
                )
                nc.sync.dma_start(
                    em0_sb[:, s:s + 1], em_in[s, 0:1, :].rearrange("o p -> p o")
                )
            nc.vector.tensor_add(t1hist[:, 0:NS], start_sb[:], em0_sb[:])

            # ---- prologue: transpose emissions into em_cols[i, t*NS+s] ----
            for s in range(NS):
                for c in range(T // S):
                    stage = stage_pool.tile([S, S], F32, tag="emstage")
                    nc.sync.dma_start(stage[:], em_in[s, c * S:(c + 1) * S, :])
                    pst = pst_pool.tile([S, S], F32, tag="empsum")
                    nc.tensor.transpose(pst[:], stage[:], ident[:])
                    dst = em_cols[:, c * S * NS + s: (c + 1) * S * NS: NS]
                    nc.scalar.copy(dst, pst[:])

            # ---- main DP loop (fully static) ----
            eng = {"v": nc.vector, "a": nc.scalar, "g": nc.gpsimd}

            def step(t):
                sc_tiles = []
                for s in range(NS):
                    sc = sc_pool.tile([S, S], F32, tag=f"sc{s}")
                    t1col = t1hist[:, (t - 1) * NS + s:(t - 1) * NS + s + 1]
                    src = trans_sb[:, s * S:(s + 1) * S]
                    if ADD_ENG[s] == "a":
                        nc.scalar.activation(
                            sc[:], src, mybir.ActivationFunctionType.Identity,
                            bias=t1col, scale=1.0,
                        )
                    else:
                        eng[ADD_ENG[s]].tensor_scalar_add(sc[:], src, t1col)
                    sc_tiles.append(sc)
                tmp = tmp_pool.tile([S, NS], F32, tag="tmp")
                pst = psum_pp[t % 3]
                for s in range(NS):
                    nc.tensor.transpose(pst[:, s * S:(s + 1) * S], sc_tiles[s][:], ident[:])
                pg = pst[:].rearrange("p (s i) -> p s i", i=S)
                nc.vector.tensor_reduce(
                    tmp[:], pg,
                    axis=mybir.AxisListType.X, op=mybir.AluOpType.max)
                nc.vector.tensor_add(
                    t1hist[:, t * NS:(t + 1) * NS],
                    tmp[:],
                    em_cols[:, t * NS:(t + 1) * NS])

            for t in range(1, T):
                step(t)

            # ---- epilogue: stream t1 history out ----
            n_dma = 8
            cols = T * NS // n_dma
            for d in range(n_dma):
                nc.sync.dma_start(
                    t1_out[:, d * cols:(d + 1) * cols],
                    t1hist[:, d * cols:(d + 1) * cols])

    nc.finalize()
    return nc


def _get_nc():
    if "nc" not in _CACHE:
        _CACHE["nc"] = _build_forward()
    return _CACHE["nc"]


def kernel(transitions, emissions, lengths):
    from concourse.bass_utils import run_bass_kernel_spmd

    transitions = np.ascontiguousarray(transitions, dtype=np.float32)
    emissions = np.ascontiguousarray(emissions, dtype=np.float32)
    lengths = np.asarray(lengths, dtype=np.int32)
    assert transitions.shape == (B, S + 1, S)
    assert emissions.shape == (B, T, S)

    nc = _get_nc()
    eye = np.eye(S, dtype=np.float32)
    in_maps = [
        {
            "transitions": transitions[c * NS:(c + 1) * NS],
            "emissions": emissions[c * NS:(c + 1) * NS],
            "identity": eye,
        }
        for c in range(N_CORES)
    ]
    res = run_bass_kernel_spmd(
        nc, in_maps, core_ids=list(range(N_CORES)),
        trace=bool(os.environ.get("VIT_TRACE")),
    )
    if os.environ.get("VIT_TRACE"):
        _CACHE["last_exec_time_ns"] = res.exec_time_ns
        _CACHE["last_result"] = res
        if res.instructions_and_trace:
            _CACHE["last_trace_path"] = res.instructions_and_trace[1]

    # t1[n, t, i], bitwise-identical to the reference's t1 trajectory for
    # t < lengths[n] (unfrozen beyond, which the backtrack never reads)
    t1 = np.empty((B, T, S), dtype=np.float32)
    for c in range(N_CORES):
        t1[c * NS:(c + 1) * NS] = (
            res.results[c]["t1hist"].reshape(S, T, NS).transpose(2, 1, 0)
        )

    return _backtrack(transitions, emissions, lengths, t1)


def _backtrack(transitions, emissions, lengths, t1):
    """Reference-exact backtrack from the t1 value history.

    bp_t[i] = argmax_j((t1_{t-1}[j] + trans[j, i]) + em[t, i]); we evaluate
    only the column i = ptr_t actually visited, with the same fp32 operation
    order and first-max tie-breaking as the reference.
    """
    trans = transitions[:, :S, :]
    nb = np.arange(B)
    z = np.zeros((B, T), dtype=np.int32)
    last = lengths - 1
    z_last = np.argmax(t1[nb, last, :], axis=1).astype(np.int32)
    ptr = z_last.copy()
    # skip all t > max(last): z is zero there for every sequence
    for t in range(int(last.max()), 0, -1):
        at_last = (t == last)
        if at_last.any():
            ptr = np.where(at_last, z_last, ptr)
        z[:, t] = np.where(t <= last, ptr, 0)
        col = (t1[:, t - 1, :] + trans[nb, :, ptr]) + emissions[nb, t, ptr][:, None]
        ptr_new = np.argmax(col, axis=1).astype(np.int32)
        ptr = np.where(t <= last, ptr_new, ptr)
    z[:, 0] = ptr
    return z



# revision 12
# speedup vs baseline: 1.4664x; 1.4664x over previous
"""Batched Viterbi decode (CRF) on 8 Trainium2 NeuronCores.

Problem (hardcoded): B=64, T=2048, S=128 tags.
  transitions [64, 129, 128] f32   (row 128 = start scores)
  emissions   [64, 2048, 128] f32
  lengths     [64] i32
  output z    [64, 2048] i32  (Viterbi path, zeros past each length)

Strategy:
  - Data-parallel over batch: core c owns sequences 8c..8c+7.
  - Device computes the forward max-plus DP *values* only (no argmax):
      t1_t[i] = max_j(t1_{t-1}[j] + trans[j,i]) + em[t,i]
    keeping the full t1 history [128, T*8] in SBUF, streamed to DRAM at the
    end. All adds/maxes are IEEE fp32 in the same association order as the
    reference, so the t1 history is bitwise identical to the reference's.
  - Host reconstructs the backpointers only along the surviving path
    (argmax_j of one 128-column per step per sequence), bitwise-reproducing
    the reference's tie-breaking, then emits z.

  Per step the device does (per core, 8 sequences):
    scores_s[j,i] = trans_s[j,i] + t1_col_s[j]   (6 seqs on ACT bias-add,
                                                  2 seqs on DVE tensor_scalar)
    scoresT_s = PE transpose(scores_s) -> PSUM  (per 4-seq group tensors)
    m[g] = DVE tensor_reduce(max) over j for each 4-seq group
    t1hist[:, t*8+g*4 : ...] = m[g] + em_cols[g]  (DVE tensor_tensor)
"""

import os
from contextlib import ExitStack

import numpy as np

S = 128
T = 2048
NS = 8          # sequences per core
N_CORES = 8
B = NS * N_CORES
ADD_ENG = "aavaavaa"   # per-seq engine for the t1 broadcast add: a=ACT, v=DVE

_CACHE = {}


def _build_forward():
    import concourse.bacc as bacc
    import concourse.mybir as mybir
    import concourse.tile as tile

    F32 = mybir.dt.float32
    nc = bacc.Bacc("TRN2", num_devices=N_CORES)
    trans_in = nc.dram_tensor("transitions", [NS, S + 1, S], F32, kind="ExternalInput")
    em_in = nc.dram_tensor("emissions", [NS, T, S], F32, kind="ExternalInput")
    ident_in = nc.dram_tensor("identity", [S, S], F32, kind="ExternalInput")
    t1_out = nc.dram_tensor("t1hist", [S, T * NS], F32, kind="ExternalOutput")

    with ExitStack() as ctx:
        trans_sb = ctx.enter_context(nc.sbuf_tensor([S, NS * S], F32))
        em_cols = ctx.enter_context(nc.sbuf_tensor([S, T * NS], F32))
        t1hist = ctx.enter_context(nc.sbuf_tensor([S, T * NS], F32))
        ident = ctx.enter_context(nc.sbuf_tensor([S, S], F32))
        start_sb = ctx.enter_context(nc.sbuf_tensor([S, NS], F32))
        em0_sb = ctx.enter_context(nc.sbuf_tensor([S, NS], F32))
        # per-(buf, group) PSUM tensors so each 4-seq group reduce only
        # depends on its own 4 transposes
        psum_pp = [
            [
                ctx.enter_context(nc.psum_tensor(f"pspp{k}g{g}", [S, NS * S // 2], F32))
                for g in range(2)
            ]
            for k in range(3)
        ]

        with tile.TileContext(nc) as tc, ExitStack() as pctx:
            sc_pool = pctx.enter_context(tc.tile_pool(name="scores", bufs=3))
            tmp_pool = pctx.enter_context(tc.tile_pool(name="tmp", bufs=3))
            stage_pool = pctx.enter_context(tc.tile_pool(name="stage", bufs=4))
            pst_pool = pctx.enter_context(tc.tile_pool(name="pst", bufs=2, space="PSUM"))

            # ---- prologue: transitions, identity, t1_0 ----
            for s in range(NS):
                nc.sync.dma_start(trans_sb[:, s * S:(s + 1) * S], trans_in[s, 0:S, :])
            nc.sync.dma_start(ident[:], ident_in[:])
            for s in range(NS):
                nc.sync.dma_start(
                    start_sb[:, s:s + 1], trans_in[s, S:S + 1, :].rearrange("o p -> p o")
                )
                nc.sync.dma_start(
                    em0_sb[:, s:s + 1], em_in[s, 0:1, :].rearrange("o p -> p o")
                )
            nc.vector.tensor_add(t1hist[:, 0:NS], start_sb[:], em0_sb[:])

            # ---- prologue: transpose emissions into em_cols[i, t*NS+s] ----
            for s in range(NS):
                for c in range(T // S):
                    stage = stage_pool.tile([S, S], F32, tag="emstage")
                    nc.sync.dma_start(stage[:], em_in[s, c * S:(c + 1) * S, :])
                    pst = pst_pool.tile([S, S], F32, tag="empsum")
                    nc.tensor.transpose(pst[:], stage[:], ident[:])
                    dst = em_cols[:, c * S * NS + s: (c + 1) * S * NS: NS]
                    nc.scalar.copy(dst, pst[:])

            # ---- main DP loop (fully static) ----
            eng = {"v": nc.vector, "a": nc.scalar, "g": nc.gpsimd}

            def step(t):
                sc_tiles = []
                for s in range(NS):
                    sc = sc_pool.tile([S, S], F32, tag=f"sc{s}")
                    t1col = t1hist[:, (t - 1) * NS + s:(t - 1) * NS + s + 1]
                    src = trans_sb[:, s * S:(s + 1) * S]
                    if ADD_ENG[s] == "a":
                        nc.scalar.activation(
                            sc[:], src, mybir.ActivationFunctionType.Identity,
                            bias=t1col, scale=1.0,
                        )
                    else:
                        eng[ADD_ENG[s]].tensor_scalar_add(sc[:], src, t1col)
                    sc_tiles.append(sc)
                tmp = tmp_pool.tile([S, NS], F32, tag="tmp")
                for g in range(2):
                    pst = psum_pp[t % 3][g]
                    for sl in range(4):
                        s = g * 4 + sl
                        nc.tensor.transpose(pst[:, sl * S:(sl + 1) * S], sc_tiles[s][:], ident[:])
                    pg = pst[:].rearrange("p (s i) -> p s i", i=S)
                    nc.vector.tensor_reduce(
                        tmp[:, g * 4:(g + 1) * 4], pg,
                        axis=mybir.AxisListType.X, op=mybir.AluOpType.max)
                    nc.vector.tensor_add(
                        t1hist[:, t * NS + g * 4:t * NS + (g + 1) * 4],
                        tmp[:, g * 4:(g + 1) * 4],
                        em_cols[:, t * NS + g * 4:t * NS + (g + 1) * 4])

            for t in range(1, T):
                step(t)

            # ---- epilogue: stream t1 history out ----
            n_dma = 8
            cols = T * NS // n_dma
            for d in range(n_dma):
                nc.sync.dma_start(
                    t1_out[:, d * cols:(d + 1) * cols],
                    t1hist[:, d * cols:(d + 1) * cols])

    nc.finalize()
    return nc


def _get_nc():
    if "nc" not in _CACHE:
        _CACHE["nc"] = _build_forward()
    return _CACHE["nc"]


def kernel(transitions, emissions, lengths):
    from concourse.bass_utils import run_bass_kernel_spmd

    transitions = np.ascontiguousarray(transitions, dtype=np.float32)
    emissions = np.ascontiguousarray(emissions, dtype=np.float32)
    lengths = np.asarray(lengths, dtype=np.int32)
    assert transitions.shape == (B, S + 1, S)
    assert emissions.shape == (B, T, S)

    nc = _get_nc()
    eye = np.eye(S, dtype=np.float32)
    in_maps = [
        {
            "transitions": transitions[c * NS:(c + 1) * NS],
            "emissions": emissions[c * NS:(c + 1) * NS],
            "identity": eye,
        }
        for c in range(N_CORES)
    ]
    res = run_bass_kernel_spmd(
        nc, in_maps, core_ids=list(range(N_CORES)),
        trace=bool(os.environ.get("VIT_TRACE")),
    )
    if os.environ.get("VIT_TRACE"):
        _CACHE["last_exec_time_ns"] = res.exec_time_ns
        _CACHE["last_result"] = res
        if res.instructions_and_trace:
            _CACHE["last_trace_path"] = res.instructions_and_trace[1]

    # t1[n, t, i], bitwise-identical to the reference's t1 trajectory for
    # t < lengths[n] (unfrozen beyond, which the backtrack never reads)
    t1 = np.empty((B, T, S), dtype=np.float32)
    for c in range(N_CORES):
        t1[c * NS:(c + 1) * NS] = (
            res.results[c]["t1hist"].reshape(S, T, NS).transpose(2, 1, 0)
        )

    return _backtrack(transitions, emissions, lengths, t1)


def _backtrack(transitions, emissions, lengths, t1):
    """Reference-exact backtrack from the t1 value history.

    bp_t[i] = argmax_j((t1_{t-1}[j] + trans[j, i]) + em[t, i]); we evaluate
    only the column i = ptr_t actually visited, with the same fp32 operation
    order and first-max tie-breaking as the reference.
    """
    trans = transitions[:, :S, :]
    nb = np.arange(B)
    z = np.zeros((B, T), dtype=np.int32)
    last = lengths - 1
    z_last = np.argmax(t1[nb, last, :], axis=1).astype(np.int32)
    ptr = z_last.copy()
    # skip all t > max(last): z is zero there for every sequence
    for t in range(int(last.max()), 0, -1):
        at_last = (t == last)
        if at_last.any():
            ptr = np.where(at_last, z_last, ptr)
        z[:, t] = np.where(t <= last, ptr, 0)
        col = (t1[:, t - 1, :] + trans[nb, :, ptr]) + emissions[nb, t, ptr][:, None]
        ptr_new = np.argmax(col, axis=1).astype(np.int32)
        ptr = np.where(t <= last, ptr_new, ptr)
    z[:, 0] = ptr
    return z



# revision 13
# speedup vs baseline: 1.5702x; 1.0707x over previous
"""Batched Viterbi decode (CRF) on 8 Trainium2 NeuronCores.

Problem (hardcoded): B=64, T=2048, S=128 tags.
  transitions [64, 129, 128] f32   (row 128 = start scores)
  emissions   [64, 2048, 128] f32
  lengths     [64] i32
  output z    [64, 2048] i32  (Viterbi path, zeros past each length)

Strategy:
  - Data-parallel over batch: core c owns sequences 8c..8c+7.
  - Device computes the forward max-plus DP *values* only (no argmax):
      t1_t[i] = max_j(t1_{t-1}[j] + trans[j,i]) + em[t,i]
    keeping the full t1 history [128, T*8] in SBUF, streamed to DRAM at the
    end. All adds/maxes are IEEE fp32 in the same association order as the
    reference, so the t1 history is bitwise identical to the reference's.
  - Host reconstructs the backpointers only along the surviving path
    (argmax_j of one 128-column per step per sequence), bitwise-reproducing
    the reference's tie-breaking, then emits z.

  Per step the device does (per core, 8 sequences):
    scores_s[j,i] = trans_s[j,i] + t1_col_s[j]   (6 seqs on ACT bias-add,
                                                  2 seqs on DVE tensor_scalar)
    scoresT_s = PE transpose(scores_s) -> PSUM  (per 4-seq group tensors)
    m[g] = DVE tensor_reduce(max) over j for each 4-seq group
    t1hist[:, t*8+g*4 : ...] = m[g] + em_cols[g]  (DVE tensor_tensor)
"""

import os
from contextlib import ExitStack

import numpy as np

S = 128
T = 2048
NS = 8          # sequences per core
N_CORES = 8
B = NS * N_CORES
ADD_ENG = "aavaavaa"   # per-seq engine for the t1 broadcast add: a=ACT, v=DVE

_CACHE = {}


def _build_forward():
    import concourse.bacc as bacc
    import concourse.mybir as mybir
    import concourse.tile as tile

    F32 = mybir.dt.float32
    nc = bacc.Bacc("TRN2", num_devices=N_CORES)
    trans_in = nc.dram_tensor("transitions", [NS, S + 1, S], F32, kind="ExternalInput")
    em_in = nc.dram_tensor("emissions", [NS, T, S], F32, kind="ExternalInput")
    ident_in = nc.dram_tensor("identity", [S, S], F32, kind="ExternalInput")
    t1_out = nc.dram_tensor("t1hist", [S, T * NS], F32, kind="ExternalOutput")

    with ExitStack() as ctx:
        trans_sb = ctx.enter_context(nc.sbuf_tensor([S, NS * S], F32))
        em_cols = ctx.enter_context(nc.sbuf_tensor([S, T * NS], F32))
        t1hist = ctx.enter_context(nc.sbuf_tensor([S, T * NS], F32))
        ident = ctx.enter_context(nc.sbuf_tensor([S, S], F32))
        start_sb = ctx.enter_context(nc.sbuf_tensor([S, NS], F32))
        em0_sb = ctx.enter_context(nc.sbuf_tensor([S, NS], F32))
        # per-(buf, group) PSUM tensors so each 4-seq group reduce only
        # depends on its own 4 transposes
        psum_pp = [
            [
                ctx.enter_context(nc.psum_tensor(f"pspp{k}g{g}", [S, NS * S // 2], F32))
                for g in range(2)
            ]
            for k in range(3)
        ]

        with tile.TileContext(nc) as tc, ExitStack() as pctx:
            sc_pool = pctx.enter_context(tc.tile_pool(name="scores", bufs=3))
            tmp_pool = pctx.enter_context(tc.tile_pool(name="tmp", bufs=3))
            stage_pool = pctx.enter_context(tc.tile_pool(name="stage", bufs=4))
            pst_pool = pctx.enter_context(tc.tile_pool(name="pst", bufs=2, space="PSUM"))

            # ---- prologue: transitions, identity, t1_0 ----
            for s in range(NS):
                nc.sync.dma_start(trans_sb[:, s * S:(s + 1) * S], trans_in[s, 0:S, :])
            nc.sync.dma_start(ident[:], ident_in[:])
            for s in range(NS):
                nc.sync.dma_start(
                    start_sb[:, s:s + 1], trans_in[s, S:S + 1, :].rearrange("o p -> p o")
                )
                nc.sync.dma_start(
                    em0_sb[:, s:s + 1], em_in[s, 0:1, :].rearrange("o p -> p o")
                )
            nc.vector.tensor_add(t1hist[:, 0:NS], start_sb[:], em0_sb[:])

            # ---- prologue: transpose emissions into em_cols[i, t*NS+s] ----
            for s in range(NS):
                for c in range(T // S):
                    stage = stage_pool.tile([S, S], F32, tag="emstage")
                    nc.sync.dma_start(stage[:], em_in[s, c * S:(c + 1) * S, :])
                    pst = pst_pool.tile([S, S], F32, tag="empsum")
                    nc.tensor.transpose(pst[:], stage[:], ident[:])
                    dst = em_cols[:, c * S * NS + s: (c + 1) * S * NS: NS]
                    nc.scalar.copy(dst, pst[:])

            # ---- main DP loop (fully static) ----
            eng = {"v": nc.vector, "a": nc.scalar, "g": nc.gpsimd}

            def step(t):
                sc_tiles = []
                for s in range(NS):
                    sc = sc_pool.tile([S, S], F32, tag=f"sc{s}")
                    t1col = t1hist[:, (t - 1) * NS + s:(t - 1) * NS + s + 1]
                    src = trans_sb[:, s * S:(s + 1) * S]
                    if ADD_ENG[s] == "a":
                        nc.scalar.activation(
                            sc[:], src, mybir.ActivationFunctionType.Identity,
                            bias=t1col, scale=1.0,
                        )
                    else:
                        eng[ADD_ENG[s]].tensor_scalar_add(sc[:], src, t1col)
                    sc_tiles.append(sc)
                tmp = tmp_pool.tile([S, NS], F32, tag="tmp")
                for g in range(2):
                    pst = psum_pp[t % 3][g]
                    for h in range(2):
                        for sl in range(2):
                            c = h * 2 + sl
                            s = g * 4 + c
                            nc.tensor.transpose(
                                pst[:, c * S:(c + 1) * S], sc_tiles[s][:], ident[:])
                        lo = g * 4 + h * 2
                        pg = pst[:, h * 2 * S:(h * 2 + 2) * S].rearrange(
                            "p (s i) -> p s i", i=S)
                        nc.vector.tensor_reduce(
                            tmp[:, lo:lo + 2], pg,
                            axis=mybir.AxisListType.X, op=mybir.AluOpType.max)
                        nc.vector.tensor_add(
                            t1hist[:, t * NS + lo:t * NS + lo + 2],
                            tmp[:, lo:lo + 2],
                            em_cols[:, t * NS + lo:t * NS + lo + 2])

            for t in range(1, T):
                step(t)

            # ---- epilogue: stream t1 history out ----
            n_dma = 8
            cols = T * NS // n_dma
            for d in range(n_dma):
                nc.sync.dma_start(
                    t1_out[:, d * cols:(d + 1) * cols],
                    t1hist[:, d * cols:(d + 1) * cols])

    nc.finalize()
    return nc


def _get_nc():
    if "nc" not in _CACHE:
        _CACHE["nc"] = _build_forward()
    return _CACHE["nc"]


def kernel(transitions, emissions, lengths):
    from concourse.bass_utils import run_bass_kernel_spmd

    transitions = np.ascontiguousarray(transitions, dtype=np.float32)
    emissions = np.ascontiguousarray(emissions, dtype=np.float32)
    lengths = np.asarray(lengths, dtype=np.int32)
    assert transitions.shape == (B, S + 1, S)
    assert emissions.shape == (B, T, S)

    nc = _get_nc()
    eye = np.eye(S, dtype=np.float32)
    in_maps = [
        {
            "transitions": transitions[c * NS:(c + 1) * NS],
            "emissions": emissions[c * NS:(c + 1) * NS],
            "identity": eye,
        }
        for c in range(N_CORES)
    ]
    res = run_bass_kernel_spmd(
        nc, in_maps, core_ids=list(range(N_CORES)),
        trace=bool(os.environ.get("VIT_TRACE")),
    )
    if os.environ.get("VIT_TRACE"):
        _CACHE["last_exec_time_ns"] = res.exec_time_ns
        _CACHE["last_result"] = res
        if res.instructions_and_trace:
            _CACHE["last_trace_path"] = res.instructions_and_trace[1]

    # t1[n, t, i], bitwise-identical to the reference's t1 trajectory for
    # t < lengths[n] (unfrozen beyond, which the backtrack never reads)
    t1 = np.empty((B, T, S), dtype=np.float32)
    for c in range(N_CORES):
        t1[c * NS:(c + 1) * NS] = (
            res.results[c]["t1hist"].reshape(S, T, NS).transpose(2, 1, 0)
        )

    return _backtrack(transitions, emissions, lengths, t1)


def _backtrack(transitions, emissions, lengths, t1):
    """Reference-exact backtrack from the t1 value history.

    bp_t[i] = argmax_j((t1_{t-1}[j] + trans[j, i]) + em[t, i]); we evaluate
    only the column i = ptr_t actually visited, with the same fp32 operation
    order and first-max tie-breaking as the reference.
    """
    trans = transitions[:, :S, :]
    nb = np.arange(B)
    z = np.zeros((B, T), dtype=np.int32)
    last = lengths - 1
    z_last = np.argmax(t1[nb, last, :], axis=1).astype(np.int32)
    ptr = z_last.copy()
    # skip all t > max(last): z is zero there for every sequence
    for t in range(int(last.max()), 0, -1):
        at_last = (t == last)
        if at_last.any():
            ptr = np.where(at_last, z_last, ptr)
        z[:, t] = np.where(t <= last, ptr, 0)
        col = (t1[:, t - 1, :] + trans[nb, :, ptr]) + emissions[nb, t, ptr][:, None]
        ptr_new = np.argmax(col, axis=1).astype(np.int32)
        ptr = np.where(t <= last, ptr_new, ptr)
    z[:, 0] = ptr
    return z



# revision 18
# speedup vs baseline: 1.5704x; 1.0002x over previous
"""Batched Viterbi decode (CRF) on 8 Trainium2 NeuronCores.

Problem (hardcoded): B=64, T=2048, S=128 tags.
  transitions [64, 129, 128] f32   (row 128 = start scores)
  emissions   [64, 2048, 128] f32
  lengths     [64] i32
  output z    [64, 2048] i32  (Viterbi path, zeros past each length)

Strategy:
  - Data-parallel over batch: core c owns sequences 8c..8c+7.
  - Device computes the forward max-plus DP *values* only (no argmax):
      t1_t[i] = max_j(t1_{t-1}[j] + trans[j,i]) + em[t,i]
    keeping the full t1 history [128, T*8] in SBUF, streamed to DRAM at the
    end. All adds/maxes are IEEE fp32 in the same association order as the
    reference, so the t1 history is bitwise identical to the reference's.
  - Host reconstructs the backpointers only along the surviving path
    (argmax_j of one 128-column per step per sequence), bitwise-reproducing
    the reference's tie-breaking, then emits z.

  Per step the device does (per core, 8 sequences):
    scores_s[j,i] = trans_s[j,i] + t1_col_s[j]   (6 seqs on ACT bias-add,
                                                  2 seqs on DVE tensor_scalar)
    scoresT_s = PE transpose(scores_s) -> PSUM  (per 4-seq group tensors)
    m[h] = DVE tensor_reduce(max) over j per 2-seq half-group (the narrow
           barrier lets each reduce start after only 2 transposes, which
           measured ~6% faster than 4-seq-group reduces)
    t1hist[:, t*8+h*2 : ...] = m[h] + em_cols[h]  (DVE tensor_tensor)
"""

import os
from contextlib import ExitStack

import numpy as np

S = 128
T = 2048
NS = 8          # sequences per core
N_CORES = 8
B = NS * N_CORES
ADD_ENG = "aavaavaa"   # per-seq engine for the t1 broadcast add: a=ACT, v=DVE

_CACHE = {}


def _build_forward():
    import concourse.bacc as bacc
    import concourse.mybir as mybir
    import concourse.tile as tile

    F32 = mybir.dt.float32
    nc = bacc.Bacc("TRN2", num_devices=N_CORES)
    trans_in = nc.dram_tensor("transitions", [NS, S + 1, S], F32, kind="ExternalInput")
    em_in = nc.dram_tensor("emissions", [NS, T, S], F32, kind="ExternalInput")
    ident_in = nc.dram_tensor("identity", [S, S], F32, kind="ExternalInput")
    t1_out = nc.dram_tensor("t1hist", [S, T * NS], F32, kind="ExternalOutput")

    with ExitStack() as ctx:
        trans_sb = ctx.enter_context(nc.sbuf_tensor([S, NS * S], F32))
        em_cols = ctx.enter_context(nc.sbuf_tensor([S, T * NS], F32))
        t1hist = ctx.enter_context(nc.sbuf_tensor([S, T * NS], F32))
        ident = ctx.enter_context(nc.sbuf_tensor([S, S], F32))
        start_sb = ctx.enter_context(nc.sbuf_tensor([S, NS], F32))
        em0_sb = ctx.enter_context(nc.sbuf_tensor([S, NS], F32))
        # per-(buf, group) PSUM tensors so each 4-seq group reduce only
        # depends on its own 4 transposes
        psum_pp = [
            [
                ctx.enter_context(nc.psum_tensor(f"pspp{k}g{g}", [S, NS * S // 2], F32))
                for g in range(2)
            ]
            for k in range(3)
        ]

        with tile.TileContext(nc) as tc, ExitStack() as pctx:
            sc_pool = pctx.enter_context(tc.tile_pool(name="scores", bufs=3))
            tmp_pool = pctx.enter_context(tc.tile_pool(name="tmp", bufs=3))
            stage_pool = pctx.enter_context(tc.tile_pool(name="stage", bufs=4))
            pst_pool = pctx.enter_context(tc.tile_pool(name="pst", bufs=2, space="PSUM"))

            # ---- prologue: transitions, identity, t1_0 ----
            for s in range(NS):
                nc.sync.dma_start(trans_sb[:, s * S:(s + 1) * S], trans_in[s, 0:S, :])
            nc.sync.dma_start(ident[:], ident_in[:])
            for s in range(NS):
                nc.sync.dma_start(
                    start_sb[:, s:s + 1], trans_in[s, S:S + 1, :].rearrange("o p -> p o")
                )
                nc.sync.dma_start(
                    em0_sb[:, s:s + 1], em_in[s, 0:1, :].rearrange("o p -> p o")
                )
            nc.vector.tensor_add(t1hist[:, 0:NS], start_sb[:], em0_sb[:])

            # ---- prologue: transpose emissions into em_cols[i, t*NS+s] ----
            for s in range(NS):
                for c in range(T // S):
                    stage = stage_pool.tile([S, S], F32, tag="emstage")
                    nc.sync.dma_start(stage[:], em_in[s, c * S:(c + 1) * S, :])
                    pst = pst_pool.tile([S, S], F32, tag="empsum")
                    nc.tensor.transpose(pst[:], stage[:], ident[:])
                    dst = em_cols[:, c * S * NS + s: (c + 1) * S * NS: NS]
                    nc.scalar.copy(dst, pst[:])

            # ---- main DP loop (fully static) ----
            eng = {"v": nc.vector, "a": nc.scalar, "g": nc.gpsimd}

            def step(t):
                sc_tiles = []
                for s in range(NS):
                    sc = sc_pool.tile([S, S], F32, tag=f"sc{s}")
                    t1col = t1hist[:, (t - 1) * NS + s:(t - 1) * NS + s + 1]
                    src = trans_sb[:, s * S:(s + 1) * S]
                    if ADD_ENG[s] == "a":
                        nc.scalar.activation(
                            sc[:], src, mybir.ActivationFunctionType.Identity,
                            bias=t1col, scale=1.0,
                        )
                    else:
                        eng[ADD_ENG[s]].tensor_scalar_add(sc[:], src, t1col)
                    sc_tiles.append(sc)
                tmp = tmp_pool.tile([S, NS], F32, tag="tmp")
                for g in range(2):
                    pst = psum_pp[t % 3][g]
                    for h in range(2):
                        for sl in range(2):
                            c = h * 2 + sl
                            s = g * 4 + c
                            nc.tensor.transpose(
                                pst[:, c * S:(c + 1) * S], sc_tiles[s][:], ident[:])
                        lo = g * 4 + h * 2
                        pg = pst[:, h * 2 * S:(h * 2 + 2) * S].rearrange(
                            "p (s i) -> p s i", i=S)
                        nc.vector.tensor_reduce(
                            tmp[:, lo:lo + 2], pg,
                            axis=mybir.AxisListType.X, op=mybir.AluOpType.max)
                        nc.vector.tensor_add(
                            t1hist[:, t * NS + lo:t * NS + lo + 2],
                            tmp[:, lo:lo + 2],
                            em_cols[:, t * NS + lo:t * NS + lo + 2])

            for t in range(1, T):
                step(t)

            # ---- epilogue: stream t1 history out ----
            n_dma = 8
            cols = T * NS // n_dma
            for d in range(n_dma):
                nc.sync.dma_start(
                    t1_out[:, d * cols:(d + 1) * cols],
                    t1hist[:, d * cols:(d + 1) * cols])

    nc.finalize()
    return nc


def _get_nc():
    if "nc" not in _CACHE:
        _CACHE["nc"] = _build_forward()
    return _CACHE["nc"]


def kernel(transitions, emissions, lengths):
    from concourse.bass_utils import run_bass_kernel_spmd

    transitions = np.ascontiguousarray(transitions, dtype=np.float32)
    emissions = np.ascontiguousarray(emissions, dtype=np.float32)
    lengths = np.asarray(lengths, dtype=np.int32)
    assert transitions.shape == (B, S + 1, S)
    assert emissions.shape == (B, T, S)

    nc = _get_nc()
    eye = np.eye(S, dtype=np.float32)
    in_maps = [
        {
            "transitions": transitions[c * NS:(c + 1) * NS],
            "emissions": emissions[c * NS:(c + 1) * NS],
            "identity": eye,
        }
        for c in range(N_CORES)
    ]
    res = run_bass_kernel_spmd(
        nc, in_maps, core_ids=list(range(N_CORES)),
        trace=bool(os.environ.get("VIT_TRACE")),
    )
    if os.environ.get("VIT_TRACE"):
        _CACHE["last_exec_time_ns"] = res.exec_time_ns
        _CACHE["last_result"] = res
        if res.instructions_and_trace:
            _CACHE["last_trace_path"] = res.instructions_and_trace[1]

    # t1[n, t, i], bitwise-identical to the reference's t1 trajectory for
    # t < lengths[n] (unfrozen beyond, which the backtrack never reads)
    t1 = np.empty((B, T, S), dtype=np.float32)
    for c in range(N_CORES):
        t1[c * NS:(c + 1) * NS] = (
            res.results[c]["t1hist"].reshape(S, T, NS).transpose(2, 1, 0)
        )

    return _backtrack(transitions, emissions, lengths, t1)


def _backtrack(transitions, emissions, lengths, t1):
    """Reference-exact backtrack from the t1 value history.

    bp_t[i] = argmax_j((t1_{t-1}[j] + trans[j, i]) + em[t, i]); we evaluate
    only the column i = ptr_t actually visited, with the same fp32 operation
    order and first-max tie-breaking as the reference.
    """
    trans = transitions[:, :S, :]
    nb = np.arange(B)
    z = np.zeros((B, T), dtype=np.int32)
    last = lengths - 1
    z_last = np.argmax(t1[nb, last, :], axis=1).astype(np.int32)
    ptr = z_last.copy()
    # skip all t > max(last): z is zero there for every sequence
    for t in range(int(last.max()), 0, -1):
        at_last = (t == last)
        if at_last.any():
            ptr = np.where(at_last, z_last, ptr)
        z[:, t] = np.where(t <= last, ptr, 0)
        col = (t1[:, t - 1, :] + trans[nb, :, ptr]) + emissions[nb, t, ptr][:, None]
        ptr_new = np.argmax(col, axis=1).astype(np.int32)
        ptr = np.where(t <= last, ptr_new, ptr)
    z[:, 0] = ptr
    return z

